# revision 1
# baseline (speedup 1.0000x reference)
"""NerfHead Trainium2 kernel: per-sample generated 2-layer MLP over pixels.

Sharding: pure data parallel over the batch dim across 8 cores.
Host does all layout permutations / dtype casts (restaging only).

Per core (B=256 samples):
  Phase 1: params j-tiles [128, B] PSUM (pair-batched ACT evac when bias
    is all-zero) into big SBUF laid out [128(i), B, NJ] so per-sample mlp
    stationaries are CONTIGUOUS (fast LDWEIGHTS). W streamed as 1.5MB
    slab-major host-prepped DMAs (12KB contiguous per partition), 3-deep,
    alternating sync/scalar HWDGE queues. cn2 via squares + masked-ones
    MM accumulation; pixel RMS stats pair-stacked into msp [128, 512].
  Normalizers: batched ACT Sqrt + DVE reciprocal (one-time ~5us).
  Phase 2 (per quad = 4 samples): 2 sbc broadcast MMs (ident-col, N=512)
    into a 2-bank PSUM quad -> ONE DVE stt xn = pp*nw*inv -> mlp1 ->
    ACT Silu(scale=inv_cn col) -> mlp2 into o-quad [128, 4, 256] PSUM ->
    ONE DVE tensor_add (+pixels residual) -> bf16 granule store.
  Pixels/out use granule-major host layout (4KB/partition descriptors).
  Output bf16; host upcasts to f32.
"""
import sys
from contextlib import ExitStack

import ml_dtypes
import numpy as np

sys.path.insert(0, "/opt/trn_rl_repo")

import concourse.bass as bass  # noqa: E402
import concourse.tile as tile  # noqa: E402
from concourse import bacc, mybir  # noqa: E402

BF16 = mybir.dt.bfloat16
F32 = mybir.dt.float32
AF = mybir.ActivationFunctionType
MULT = mybir.AluOpType.mult

RMS_EPS = 1.1920928955078125e-07

N_CORES = 8
BS = 2048
NPIX = 256
D = 128
PD = 768  # patch_dim
KC = PD // 128  # 6 contraction chunks
NJ = 2 * D  # 256 j-tiles total (128 per half)
JG = 8  # j-tiles per W mega-slab DMA (1.5MB each)
PIXG = 8  # samples per pixel DMA granule (= 2 quads)


def build_program(B, zero_bias):
    """Build the per-core Bass program for a shard of B samples."""
    assert B % PIXG == 0 and B <= 256
    npair = B // 2
    nquad = B // 4
    ngran = B // PIXG
    nc = bacc.Bacc("TRN2", target_bir_lowering=False, debug=False,
                   num_devices=N_CORES)

    pixG_d = nc.dram_tensor("pixG", (ngran, D, PIXG, NPIX), BF16,
                            kind="ExternalInput")
    patG_d = nc.dram_tensor("patG", (128, KC, B), BF16, kind="ExternalInput")
    w_d = nc.dram_tensor("W", (2, D // JG, 128, KC * JG * 128), BF16,
                         kind="ExternalInput")
    bias_d = nc.dram_tensor("Bias", (2, D, D), F32, kind="ExternalInput")
    nwc_d = nc.dram_tensor("normwc", (D, 1), F32, kind="ExternalInput")
    id_d = nc.dram_tensor("ident", (D, D), BF16, kind="ExternalInput")
    outG_d = nc.dram_tensor("outG", (ngran, D, PIXG, NPIX), BF16,
                            kind="ExternalOutput")

    with tile.TileContext(nc) as tc, ExitStack() as ctx:
        const = ctx.enter_context(tc.tile_pool(name="const", bufs=1))
        bigp = ctx.enter_context(tc.tile_pool(name="big", bufs=1))

        # constants / persistent tiles. Only patches gate the first
        # matmul; the small constants load on gpsimd (SWDGE) so the
        # HWDGE queues go straight to the first W slab.
        pats = const.tile([128, KC, B], BF16, tag="pat")
        nc.sync.dma_start(pats[:], patG_d.ap())
        bt = const.tile([D, 2, D], F32, tag="bias")
        nc.gpsimd.dma_start(bt[:], bias_d.ap().rearrange("h i j -> i h j"))
        nwc = const.tile([D, 1], F32, tag="normwc")
        nc.gpsimd.dma_start(nwc[:], nwc_d.ap())
        ident = const.tile([D, D], BF16, tag="ident")
        nc.gpsimd.dma_start(ident[:], id_d.ap())
        maskones = const.tile([128, 2 * D + 1], BF16, tag="maskones")
        nc.vector.memset(maskones[:], 0.0)
        nc.vector.memset(maskones[:, D:D + 1], 1.0)
        epsb = const.tile([128, 1], F32, tag="epsb")
        nc.vector.memset(epsb[:], RMS_EPS)
        inv_cn = const.tile([D, B], F32, tag="invcn")
        cn_tmp = const.tile([D, B], F32, tag="cntmp")
        invp = const.tile([128, 2 * NPIX], BF16, tag="invp")
        rms_t = const.tile([128, 2 * NPIX], F32, tag="rmst")

        # big laid out [i, s, jj]: per-sample stationaries contiguous
        big = bigp.tile([128, B, NJ], BF16, tag="big")

        # ---- Phase 1: params gen + cn2 + pair-stacked pixel RMS stats ----
        with tc.tile_pool(name="wslab", bufs=3) as wpool, \
             tc.tile_pool(name="sq1", bufs=6) as sqpool, \
             tc.tile_pool(name="pixa", bufs=3) as pixap, \
             tc.tile_pool(name="sqp", bufs=6) as sqpp, \
             tc.tile_pool(name="mm1ps", bufs=5, space="PSUM") as mm1ps, \
             tc.tile_pool(name="cn2ps", bufs=1, space="PSUM") as cn2ps, \
             tc.tile_pool(name="msps", bufs=1, space="PSUM") as msps:
            cn2 = cn2ps.tile([D, B], F32, tag="cn2")
            msp = msps.tile([128, 2 * NPIX], F32, tag="msp")
            pending_cn2 = []
            pending_ms = []
            pixa_cur = [None]

            def emit_cn2(j, sq):
                nc.tensor.matmul(cn2[:], maskones[:, D - j:2 * D - j], sq[:],
                                 start=(j == 0), stop=(j == D - 1))

            def emit_ms(t, sqp):
                nc.tensor.matmul(msp[:],
                                 maskones[:, D - t:2 * D - t], sqp[:],
                                 start=(t == 0), stop=(t == npair - 1))

            def stats_stage(t):
                # pair t (samples 2t, 2t+1): granule load every PIXG/2
                # pairs (gpsimd SWDGE; HWDGE queues busy with W), square
                # the pair on DVE, delayed masked-ones ms-MM.
                gi = t % (PIXG // 2)
                if gi == 0:
                    pp = pixap.tile([128, PIXG, NPIX], BF16, tag="pixa")
                    nc.gpsimd.dma_start(pp[:],
                                        pixG_d.ap()[t // (PIXG // 2)])
                    pixa_cur[0] = pp
                sqp = sqpp.tile([128, 2, NPIX], BF16, tag="sqp")
                nc.vector.tensor_mul(sqp[:],
                                     pixa_cur[0][:, 2 * gi:2 * gi + 2, :],
                                     pixa_cur[0][:, 2 * gi:2 * gi + 2, :])
                pending_ms.append((t, sqp))
                if len(pending_ms) > 4:
                    emit_ms(*pending_ms.pop(0))

            jt = 0  # j-tile counter for stats interleave
            for half in range(2):
                for jg in range(D // JG):
                    sl = wpool.tile([128, KC, JG * 128], BF16, tag="wslab")
                    src_ap = w_d.ap()[half, jg].rearrange(
                        "p (k j) -> p k j", k=KC)
                    if half == 0 and jg == 0:
                        # split first slab across both HWDGE queues so
                        # the first matmuls start sooner
                        nc.sync.dma_start(sl[:, 0:KC // 2, :],
                                          src_ap[:, 0:KC // 2, :])
                        nc.scalar.dma_start(sl[:, KC // 2:, :],
                                            src_ap[:, KC // 2:, :])
                    else:
                        dma_eng = nc.sync if jg % 2 == 0 else nc.scalar
                        dma_eng.dma_start(sl[:], src_ap)
                    for jp in range(JG // 2):  # j-tile pairs
                        j0 = jg * JG + 2 * jp
                        jj0 = half * D + j0
                        ps = mm1ps.tile([D, 2, B], F32, tag="mm1")
                        for u in range(2):
                            for k in range(KC):
                                nc.tensor.matmul(
                                    ps[:, u, :],
                                    sl[:, k, (2 * jp + u) * 128:
                                       (2 * jp + u + 1) * 128],
                                    pats[:, k, :], start=(k == 0),
                                    stop=(k == KC - 1))
                        # evac to big[:, :, jj0:jj0+2]: dst in natural
                        # order (j-pairs contiguous -> full-word bf16
                        # writes, no RMW); source PSUM AP transposed
                        if zero_bias:
                            nc.scalar.activation(
                                big[:, :, jj0:jj0 + 2],
                                ps[:].rearrange("p u s -> p s u"),
                                AF.Identity)
                        else:
                            for u in range(2):
                                nc.scalar.activation(
                                    big[:, :, jj0 + u], ps[:, u, :],
                                    AF.Identity,
                                    bias=bt[:, half, j0 + u:j0 + u + 1])
                        if half == 0:
                            # square on ACT (one PSUM read; Square is in
                            # every act table). With nonzero bias the
                            # square must see post-bias values (big).
                            sq = sqpool.tile([D, 2, B], BF16, tag="sq")
                            if zero_bias:
                                nc.scalar.activation(sq[:], ps[:],
                                                     AF.Square)
                            else:
                                for u in range(2):
                                    nc.vector.tensor_mul(
                                        sq[:, u, :], big[:, :, jj0 + u],
                                        big[:, :, jj0 + u])
                            pending_cn2.append((j0, sq[:, 0, :]))
                            pending_cn2.append((j0 + 1, sq[:, 1, :]))
                            while len(pending_cn2) > 4:
                                emit_cn2(*pending_cn2.pop(0))
                        for u in range(2):
                            if jt < npair:
                                stats_stage(jt)
                            jt += 1
                        if jt == 140:
                            # stats + cn2 fully emitted; flush pendings
                            # and compute normalizers here so the ACT
                            # sqrts + table switches hide under half-1
                            # evacuations instead of the phase boundary
                            for args in pending_cn2:
                                emit_cn2(*args)
                            pending_cn2 = []
                            for args in pending_ms:
                                emit_ms(*args)
                            pending_ms = []
                            nc.scalar.activation(rms_t[:], msp[:], AF.Sqrt,
                                                 bias=epsb[:],
                                                 scale=1.0 / D)
                            with nc.allow_low_precision("bf16 inv-rms"):
                                nc.vector.reciprocal(invp[:], rms_t[:])
                            nc.scalar.activation(cn_tmp[:], cn2[:],
                                                 AF.Sqrt)
                            nc.vector.tensor_scalar_max(cn_tmp[:],
                                                        cn_tmp[:], 1e-12)
                            nc.vector.reciprocal(inv_cn[:], cn_tmp[:])
            assert not pending_cn2 and not pending_ms

        # ---- Phase 2: per-quad MLP pipeline ----
        with tc.tile_pool(name="pix", bufs=6) as pixp, \
             tc.tile_pool(name="xn", bufs=4) as xnp, \
             tc.tile_pool(name="sh", bufs=5) as shp, \
             tc.tile_pool(name="ot", bufs=4) as otp, \
             tc.tile_pool(name="sbcps", bufs=2, space="PSUM") as sbcps, \
             tc.tile_pool(name="hps", bufs=2, space="PSUM") as hpsp, \
             tc.tile_pool(name="ops", bufs=1, space="PSUM") as opsp:

            stA = {}
            stB = {}
            stC = {}
            pix_cur = {}
            ot_cur = {}

            def quad_pix(q):  # [128, 4, 256] slice of the granule
                pp = pix_cur[q // 2]
                qi = q % 2
                return pp[:, 4 * qi:4 * qi + 4, :]

            def load_gran(g):
                pp = pixp.tile([128, PIXG, NPIX], BF16, tag="pix")
                nc.sync.dma_start(pp[:], pixG_d.ap()[g])
                pix_cur[g] = pp

            def stage_a(q):  # inv-rms broadcast MMs for quad q
                sbc = sbcps.tile([D, 2, 2 * NPIX], F32, tag="sbc")
                for i in range(2):
                    t = 2 * q + i
                    nc.tensor.matmul(
                        sbc[:, i, :],
                        ident[:, t:t + 1].to_broadcast((D, D)), invp[:])
                stA[q] = sbc

            def stage_b(q):  # xn = pp * nw * inv, one quad-wide DVE stt
                sbc = stA.pop(q)
                xn = xnp.tile([D, 4, NPIX], BF16, tag="xn")
                nc.vector.scalar_tensor_tensor(
                    xn[:], quad_pix(q), nwc[:],
                    sbc[:].rearrange("p i (u n) -> p (i u) n", u=2),
                    op0=MULT, op1=MULT)
                stB[q] = xn

            def stage_c(q):  # mlp1 + silu per sample
                xn = stB.pop(q)
                shs = shp.tile([D, 4, NPIX], BF16, tag="sh")
                hs = []
                for v in range(2):  # pair within quad
                    h = hpsp.tile([D, 2, NPIX], F32, tag="h")
                    for u in range(2):
                        s = 4 * q + 2 * v + u
                        nc.tensor.matmul(h[:, u, :], big[:, s, 0:D],
                                         xn[:, 2 * v + u, :])
                    hs.append(h)
                for v in range(2):
                    for u in range(2):
                        s = 4 * q + 2 * v + u
                        nc.scalar.activation(shs[:, 2 * v + u, :],
                                             hs[v][:, u, :], AF.Silu,
                                             scale=inv_cn[:, s:s + 1])
                stC[q] = shs

            def stage_d(q):  # mlp2 + residual + evac, quad granularity
                shs = stC.pop(q)
                if q % 2 == 0:
                    ot = otp.tile([128, PIXG, NPIX], BF16, tag="ot",
                                  name="ot")
                    ot_cur[q // 2] = ot
                ot = ot_cur[q // 2]
                qi = q % 2
                o = opsp.tile([D, 4, NPIX], F32, tag="o")
                for v in range(4):
                    s = 4 * q + v
                    nc.tensor.matmul(o[:, v, :], big[:, s, D:2 * D],
                                     shs[:, v, :])
                nc.vector.tensor_add(
                    ot[:, 4 * qi:4 * qi + 4, :], o[:], quad_pix(q))
                if qi == 1:
                    nc.sync.dma_start(outG_d.ap()[q // 2], ot[:])

            load_gran(0)
            for q in range(nquad + 3):
                # mlp work first so the in-order PE queue never stalls
                # behind the DVE stt that frees the sbc buffer
                if 2 <= q < nquad + 2:
                    stage_c(q - 2)
                if q >= 3:
                    stage_d(q - 3)
                if 1 <= q < nquad + 1:
                    stage_b(q - 1)
                if q % 2 == 0 and 1 <= (q + 2) // 2 < nquad // 2:
                    load_gran((q + 2) // 2)
                if q < nquad:
                    stage_a(q)

    nc.compile()
    return nc


def host_prep(pixels, patches, W_pg, b_pg, norm_w):
    bf = ml_dtypes.bfloat16
    # pixels (BS, NPIX, D) -> granule-major (BS//PIXG, D, PIXG, NPIX):
    # 4KB contiguous per partition per granule
    pixG = np.ascontiguousarray(
        pixels.reshape(BS // PIXG, PIXG, NPIX, D).transpose(0, 3, 1, 2)
        .astype(bf))
    # patches (BS, PD) -> (128, KC, BS)
    patG = np.ascontiguousarray(
        patches.T.reshape(KC, 128, BS).transpose(1, 0, 2).astype(bf))
    # W_pg (2*D*D, PD): layer[i, j] = W_pg[half*D*D + i*128 + j, :]
    # device j-tile stationary needs [k-part(128), j-in-tile -> i rows]
    # slab-major: (2, D//JG, 128(p), KC, JG, 128(i)) flattened last 3
    Wp = W_pg.reshape(2, D, D, PD).transpose(0, 3, 2, 1)   # (2, PD, j, i)
    Wp = Wp.reshape(2, KC, 128, D // JG, JG, D)            # (2,k,p,jg,jl,i)
    Wp = np.ascontiguousarray(Wp.transpose(0, 3, 2, 1, 4, 5))  # 2,jg,p,k,jl,i
    Wh = Wp.reshape(2, D // JG, 128, KC * JG * 128).astype(bf)
    Bias = np.ascontiguousarray(b_pg.reshape(2, D, D)).astype(np.float32)
    nwc = np.ascontiguousarray(norm_w.reshape(D, 1)).astype(np.float32)
    ident = np.eye(D, dtype=bf)
    return pixG, patG, Wh, Bias, nwc, ident


_NC_CACHE = {}


def _run(pixels, patches, W_pg, b_pg, norm_w, **spmd_kwargs):
    from concourse.bass_utils import run_bass_kernel_spmd

    pixG, patG, Wh, Bias, nwc, ident = host_prep(
        pixels, patches, W_pg, b_pg, norm_w)
    B = pixels.shape[0] // N_CORES
    zero_bias = not np.any(b_pg)
    key = (B, zero_bias)
    if key not in _NC_CACHE:
        _NC_CACHE[key] = build_program(B, zero_bias)
    nc = _NC_CACHE[key]

    gpc = B // PIXG  # granules per core
    in_maps = []
    for c in range(N_CORES):
        in_maps.append({
            "pixG": pixG[c * gpc:(c + 1) * gpc],
            "patG": np.ascontiguousarray(patG[:, :, c * B:(c + 1) * B]),
            "W": Wh,
            "Bias": Bias,
            "normwc": nwc,
            "ident": ident,
        })
    try:
        res = run_bass_kernel_spmd(nc, in_maps, list(range(N_CORES)),
                                   **spmd_kwargs)
    except Exception:
        # transient device wedge (NRT_EXEC_UNIT_UNRECOVERABLE) — retry once
        res = run_bass_kernel_spmd(nc, in_maps, list(range(N_CORES)),
                                   **spmd_kwargs)
    outG = np.concatenate([res.results[c]["outG"] for c in range(N_CORES)], 0)
    # (BS//PIXG, D, PIXG, NPIX) -> (BS, NPIX, D) f32
    out = np.ascontiguousarray(
        outG.astype(np.float32).transpose(0, 2, 3, 1).reshape(BS, NPIX, D))
    return out, res


def kernel(pixels, patches, W_pg, b_pg, norm_w):
    out, _ = _run(pixels, patches, W_pg, b_pg, norm_w)
    return out


if __name__ == "__main__":
    rng = np.random.default_rng(0)
    inputs = {
        "pixels": rng.standard_normal((BS, NPIX, D), dtype=np.float32),
        "patches": rng.standard_normal((BS, PD), dtype=np.float32),
        "W_pg": (rng.standard_normal((2 * D * D, PD)) * 0.02).astype(np.float32),
        "b_pg": np.zeros((2 * D * D,), np.float32),
        "norm_w": np.ones((D,), np.float32),
    }
    out = kernel(**inputs)
    print(out.shape, out.dtype)

